# revision 12
# baseline (speedup 1.0000x reference)
"""Trainium2 Bass kernel for nn_DepthCalibration (symmetric version).

Math (per batch b, one NeuronCore each):
  s      = conv1d(pred*g, w, pad=1) + cb          (host, trivial)
  e[n,m] = 4 r_n.r_m - 2|r_n|^2 - 2|r_m|^2        (= -|r_n-r_m|^2 / (2*0.5^2))
  out[n] = clip(sum_m exp(e[n,m]) * s[m], 0.1, 100)

W = exp(e) is symmetric, so only the block-upper-triangle is computed:
tile T_j = W[m in block j (128 partitions), n in [128j, 4096) (free dim)],
exp-arguments from a rank-5 PE matmul (stationary = B-block of 128 m's,
moving = A rows over n), exp on ScalarE into fp16 SBUF tiles. Each tile
then feeds BOTH reductions:
  - PE matvec, stationary s_j [128,1]: contributes sum_{m in j} W[m,n]s[m]
    to out[n] for all n >= 128j, accumulated across j in a PSUM bank
    (chunk c of 512 outputs parks at partition c of the accum bank).
  - DVE scalar_tensor_tensor row-sum vs partition-broadcast s (strict
    upper columns only): contributes sum_{n>128(j+1)} W[m,n]s[n] to
    out[m] for m in block j, accumulated per-instruction via accum_out.
Host adds the two partial outputs and clips. Halves the ScalarE exp work
(the 1 elem/lane/cycle floor) vs the full-matrix kernel; PE carries the
e-matmuls (67.6k moving cols) + matvecs (67.6k cols), DVE carries the
row-sums (63.5k lane-cols).
"""

import sys

sys.path.insert(0, "/opt/trn_rl_repo")

import numpy as np

from concourse import bass, mybir
from concourse import bacc
from concourse import tile
from concourse.bass_utils import run_bass_kernel_spmd

B, N = 8, 4096
NB = N // 128            # 32 m-blocks of 128
KAUG = 5                 # rank of the augmented exponent factorization
MM = 512                 # max moving free dim per matmul
MIN_DEPTH, MAX_DEPTH = 0.1, 100.0

# matvec accumulator: matmul PSUM dests must sit at base partition 0/32/64,
# so the [1, 4096] accumulator row lives in one 3-bank PSUM tile as three
# slots: n in [0,1536) at partition 0, [1536,3072) at partition 32,
# [3072,4096) at partition 64.  (512-grid chunk c -> slot c//3, offset.)
def _acc_slot(c):
    if c < 3:
        return 0, MM * c
    if c < 6:
        return 32, MM * (c - 3)
    return 64, MM * (c - 6)

F32 = mybir.dt.float32
F32R = mybir.dt.float32r
FP16 = mybir.dt.float16

SKIP_MM = False          # ablation: drop e-matmuls
SKIP_EXP = False         # ablation: drop ACT exp
SKIP_MV = False          # ablation: drop PE matvecs
SKIP_STT = False         # ablation: drop DVE row-sums


def build_program(repeat=1):
    nc = bacc.Bacc(
        "TRN2",
        target_bir_lowering=False,
        debug=False,
        enable_asserts=False,
        num_devices=8,
    )

    A_d = nc.dram_tensor("A", (KAUG, N), F32, kind="ExternalInput").ap()
    B_d = nc.dram_tensor("Bm", (KAUG, N), F32, kind="ExternalInput").ap()
    srow_d = nc.dram_tensor("s_row", (N,), FP16, kind="ExternalInput").ap()
    svert_d = nc.dram_tensor("s_vert", (128, NB), FP16, kind="ExternalInput").ap()
    rs_d = nc.dram_tensor("rs_out", (N,), F32, kind="ExternalOutput").ap()
    acc_d = nc.dram_tensor("acc_out", (3 * 1536,), F32, kind="ExternalOutput").ap()

    AF = mybir.ActivationFunctionType
    OP = mybir.AluOpType

    from contextlib import ExitStack

    with tile.TileContext(nc) as tc, ExitStack() as stk:
        if repeat > 1:
            ET = mybir.EngineType
            stk.enter_context(
                tc.For_i(
                    0,
                    repeat,
                    1,
                    hint_engines=(ET.PE, ET.DVE, ET.Activation, ET.SP, ET.Pool),
                )
            )
        with (
            tc.tile_pool(name="const", bufs=1) as cpool,
            tc.tile_pool(name="w", bufs=3) as wpool,
            tc.tile_pool(name="sc", bufs=2) as spool,
            tc.tile_pool(name="accp", bufs=1, space="PSUM") as apool,
            tc.tile_pool(name="psa", bufs=1, space="PSUM") as ppool_a,
            tc.tile_pool(name="psb", bufs=1, space="PSUM") as ppool_b,
        ):
            # ---------------- inputs -----------------------------------
            A = cpool.tile([KAUG, N], F32R)
            Bm = cpool.tile([KAUG, N], F32R)
            sv = cpool.tile([128, NB], FP16)     # s, vertical (col j = block j)
            s_bc = cpool.tile([128, N], FP16)    # s broadcast over partitions
            rs = cpool.tile([128, NB], F32)      # row-sum accumulators
            nc.sync.dma_start(A[:], A_d[:, :].bitcast(F32R))
            nc.sync.dma_start(Bm[:], B_d[:, :].bitcast(F32R))
            nc.sync.dma_start(sv[:], svert_d[:, :])
            for q in range(4):
                sl = slice(q * (N // 4), (q + 1) * (N // 4))
                nc.sync.dma_start(
                    s_bc[:, sl],
                    srow_d[sl].rearrange("(o n) -> o n", o=1).broadcast_to(
                        (128, N // 4)
                    ),
                )
            nc.vector.memset(rs[:, NB - 1 : NB], 0.0)

            acc = apool.tile([96, 3 * MM], F32)  # matvec accumulators (3 banks)
            # only partitions 0/32/64 are matmul-written; zero the rest so
            # the full-height drain copy below reads defined data
            nc.vector.memset(acc[:], 0.0)

            # ---------------- main loop --------------------------------
            # Emission order software-pipelines PE: e-matmuls for block j,
            # then matvecs for block j-1 (whose exp has finished by then).
            def matvecs(j):
                if SKIP_MV:
                    return
                n0 = 128 * j
                wt = wtiles[j]
                g = n0
                while g < N:
                    c = g // MM
                    e = min(MM * (c + 1), N)
                    p, off = _acc_slot(c)
                    nc.tensor.matmul(
                        acc[p : p + 1, off + g - MM * c : off + e - MM * c],
                        sv[:, j : j + 1],
                        wt[:, g - n0 : e - n0],
                        start=(j == 0),
                        stop=(j == min(NB - 1, (e - 1) // 128)),
                    )
                    g = e

            # e-tile psum chunks round-robin between a 1536-wide (3-bank)
            # and a 1024-wide (2-bank) pool
            echunks = [(ppool_a, 1536), (ppool_b, 1024)]
            ecount = 0

            wtiles = {}
            for j in range(NB):
                n0 = 128 * j
                cols = N - n0
                wt = wpool.tile([128, N], FP16, tag="w")
                wtiles[j] = wt
                t0 = 0
                while t0 < cols:
                    pool, esz = echunks[ecount % 2]
                    ecount += 1
                    tlen = min(esz, cols - t0)
                    pt = pool.tile([128, esz], F32, tag="ps")
                    if not SKIP_MM:
                        for u0 in range(0, tlen, MM):
                            ulen = min(MM, tlen - u0)
                            nc.tensor.matmul(
                                pt[:, u0 : u0 + ulen],
                                Bm[:, n0 : n0 + 128],
                                A[:, n0 + t0 + u0 : n0 + t0 + u0 + ulen],
                            )
                    if not SKIP_EXP:
                        nc.scalar.activation(
                            wt[:, t0 : t0 + tlen], pt[:, :tlen], AF.Exp
                        )
                    else:
                        nc.vector.memset(wt[:, t0 : t0 + 2], 0.5)
                    t0 += tlen
                if j > 0:
                    matvecs(j - 1)
                # weighted row-sum over strict-upper columns
                if j < NB - 1 and not SKIP_STT:
                    sc = spool.tile([128, N - 128], FP16, tag="sc")
                    nc.vector.scalar_tensor_tensor(
                        sc[:, : cols - 128],
                        wt[:, 128:cols],
                        0.0,
                        s_bc[:, n0 + 128 : N],
                        OP.bypass,
                        OP.mult,
                        accum_out=rs[:, j : j + 1],
                    )
                elif SKIP_STT and j < NB - 1:
                    nc.vector.memset(rs[:, j : j + 1], 0.5)
            matvecs(NB - 1)

            # ---------------- outputs ----------------------------------
            nc.sync.dma_start(rs_d.rearrange("(c p) -> p c", p=128), rs[:])
            acc_sb = cpool.tile([96, 3 * MM], F32)
            nc.vector.tensor_copy(acc_sb[:], acc[:])
            acc_dv = acc_d.rearrange("(t o) -> t o", t=3)
            for t in range(3):
                nc.sync.dma_start(acc_dv[t : t + 1, :], acc_sb[32 * t : 32 * t + 1, :])

    nc.compile()
    return nc


_cache = {}


def _get_program(repeat=1):
    if repeat not in _cache:
        _cache[repeat] = build_program(repeat=repeat)
    return _cache[repeat]


def make_in_maps(pred_depth, ray_3d, conv_w, conv_b, global_scale):
    """Host prep: conv-smoothed s, augmented factor matrices A/B."""
    pred_depth = np.asarray(pred_depth, np.float64)
    ray_3d = np.asarray(ray_3d, np.float64)
    g = float(np.asarray(global_scale).reshape(-1)[0])
    w = np.asarray(conv_w, np.float64).reshape(-1)
    cb = float(np.asarray(conv_b).reshape(-1)[0])

    scaled = pred_depth * g                       # (B, N)
    pp = np.zeros((B, N + 2))
    pp[:, 1 : N + 1] = scaled
    s = w[0] * pp[:, :N] + w[1] * pp[:, 1 : N + 1] + w[2] * pp[:, 2:] + cb

    in_maps = []
    for b in range(B):
        r = ray_3d[b]                             # (N, 3)
        sq = np.sum(r * r, axis=1)
        A = np.empty((KAUG, N), np.float32)
        A[0:3] = r.T
        A[3] = sq
        A[4] = 1.0
        Bmat = np.empty((KAUG, N), np.float32)
        Bmat[0:3] = 4.0 * r.T
        Bmat[3] = -2.0
        Bmat[4] = -2.0 * sq
        sb = s[b].astype(np.float32)
        in_maps.append(
            {
                "A": A,
                "Bm": Bmat,
                "s_row": sb.astype(np.float16),
                "s_vert": np.ascontiguousarray(
                    sb.astype(np.float16).reshape(NB, 128).T
                ),
            }
        )
    return in_maps


def finish_outputs(res):
    out = np.empty((B, N), np.float32)
    for b in range(B):
        tot = res.results[b]["rs_out"] + res.results[b]["acc_out"][:N]
        out[b] = np.clip(tot, MIN_DEPTH, MAX_DEPTH)
    return out


def kernel(pred_depth, ray_3d, conv_w, conv_b, global_scale, repeat=1):
    nc = _get_program(repeat=repeat)
    in_maps = make_in_maps(pred_depth, ray_3d, conv_w, conv_b, global_scale)
    res = _run_with_retry(nc, in_maps)
    return finish_outputs(res)


def _run_with_retry(nc, in_maps, tries=3):
    # The shared axon device occasionally reports a transient
    # NRT_EXEC_UNIT_UNRECOVERABLE after a prior process crashed; it
    # recovers within ~20s. Retry rather than failing the whole call.
    import time as _time

    for attempt in range(tries):
        try:
            return run_bass_kernel_spmd(nc, in_maps, core_ids=list(range(B)))
        except Exception:
            if attempt == tries - 1:
                raise
            _time.sleep(25)
